# revision 1
# baseline (speedup 1.0000x reference)
"""Trainium2 Bass kernel for nn_MultiHeadAttention_79130477461654.

The reference einsum "nhqk,nhvd->nhqd" contracts k and v independently, so
out = (sum_k softmax(energy))*(sum_s v) = broadcast(sum_s v) since softmax
rows sum to 1.  With v = split_heads(x @ Wv) and the reference's direct
(n,h,q,d)->(n,s,e) reshape, the full output reduces to

    xs[n]    = sum_s x[n,s,:]                       (1024,)
    Sfull[n] = xs[n] @ Wv                           (1024,)
    WoSum    = sum_m Wo[64m+d, :]  (d=0..63)        (64, 1024)
    T[n,h,:] = Sfull[n][64h:64h+64] @ WoSum + bo    (16, 1024)
    out[n, 64h+r, :] = T[n,h,:]   for r in 0..63

numerically within ~1e-4 of the reference (softmax-row-sum rounding +
fp32r matmul rounding).  Sharding: data parallel over batch N=8, one
batch per core; Wv/Wo replicated.  All arithmetic on-device; reductions
run on the PE array chasing the DMA arrivals.
"""

import numpy as np

N, S, E, H, D = 8, 1024, 1024, 16, 64
NCORES = 8
P = 128  # partitions


def build_nc():
    import concourse.bacc as bacc
    import concourse.mybir as mybir
    from concourse.tile import TileContext

    F32 = mybir.dt.float32
    F32R = mybir.dt.float32r
    nc = bacc.Bacc("TRN2", target_bir_lowering=False, debug=False)

    xd = nc.declare_dram_parameter("x", [S, E], F32, isOutput=False)
    wvd = nc.declare_dram_parameter("Wv", [E, E], F32, isOutput=False)
    wod = nc.declare_dram_parameter("Wo", [E, E], F32, isOutput=False)
    bod = nc.declare_dram_parameter("bo128", [P, E], F32, isOutput=False)
    onesd = nc.declare_dram_parameter("ones128", [P, 1], F32, isOutput=False)
    dbld = nc.declare_dram_parameter("dblI", [P, D], F32, isOutput=False)
    outd = nc.declare_dram_parameter("out", [S, E], F32, isOutput=True)

    # two HWDGE queues: SP (sync) and ACT (scalar)
    dmae = [nc.sync, nc.scalar]

    # DRAM-side views pairing two 128-row chunks per 1 MB transfer:
    # paired(src, i)[p, c, :] = src[(2i + c)*128 + p, :]
    def paired(dram, i):
        return dram.rearrange("(i c p) e -> i p c e", p=P, c=2)[i]

    with TileContext(nc) as tc:
        with (
            tc.tile_pool(name="xin", bufs=4) as xp,
            tc.tile_pool(name="wv", bufs=4) as wvp,
            tc.tile_pool(name="wo", bufs=4) as wop,
            tc.tile_pool(name="small", bufs=1) as sp,
            tc.tile_pool(name="outsb", bufs=2) as op,
            tc.tile_pool(name="psA", bufs=1, space="PSUM") as psA,
            tc.tile_pool(name="psS", bufs=1, space="PSUM") as psS,
            tc.tile_pool(name="psF", bufs=1, space="PSUM") as psF,
            tc.tile_pool(name="psO", bufs=3, space="PSUM") as psO,
        ):
            ones_sb = sp.tile([P, 1], F32)
            dmae[0].dma_start(out=ones_sb[:], in_=onesd[:])
            dbl_sb = sp.tile([P, D], F32)
            dmae[1].dma_start(out=dbl_sb[:], in_=dbld[:])
            bo_sb = sp.tile([P, E], F32)
            dmae[1].dma_start(out=bo_sb[:], in_=bod[:])
            dbl_r = sp.tile([P, D], F32R)
            nc.vector.tensor_copy(dbl_r[:], dbl_sb[:])

            # ---- input DMAs: x, Wv, Wo as 1 MB paired transfers, 2 per queue
            #      x tiles reduce pairwise on DVE (chasing the DMAs):
            #      xacc2[p, cp*E + e] = sum_i x[(2i+cp)*128 + p, e]
            xacc2 = sp.tile([P, 2 * E], F32)
            wvt, wot = [], []
            for i in range(4):
                t = xp.tile([P, 2 * E], F32)
                dmae[i % 2].dma_start(
                    out=t[:].rearrange("p (c e) -> p c e", c=2), in_=paired(xd, i)
                )
                if i == 0:
                    nc.vector.tensor_copy(xacc2[:], t[:])
                else:
                    nc.vector.tensor_add(xacc2[:], xacc2[:], t[:])
            for i in range(4):
                t = wvp.tile([P, 2 * E], F32, tag="wvf")
                dmae[i % 2].dma_start(
                    out=t[:].rearrange("p (c e) -> p c e", c=2), in_=paired(wvd, i)
                )
                tr = wvp.tile([P, 2 * E], F32R, tag="wvr")
                nc.vector.tensor_copy(tr[:], t[:])
                wvt.append(tr)
            for i in range(4):
                t = wop.tile([P, 2 * E], F32, tag="wof")
                dmae[i % 2].dma_start(
                    out=t[:].rearrange("p (c e) -> p c e", c=2), in_=paired(wod, i)
                )
                tr = wop.tile([P, 2 * E], F32R, tag="wor")
                nc.vector.tensor_copy(tr[:], t[:])
                wot.append(tr)

            # ---- xsT[p, c] = xs[128c + p]: PE partition-reduction.
            #      Per-column groups are CONTIGUOUS (a start=True clears the
            #      whole PSUM bank's has_written, so groups sharing a bank
            #      must not interleave).
            ps_xsT = psA.tile([P, 8], F32, tag="psa")
            for c in range(8):
                for cp in range(2):
                    nc.tensor.matmul(
                        ps_xsT[:, c : c + 1],
                        xacc2[:, cp * E + c * P : cp * E + (c + 1) * P],
                        ones_sb[:],
                        start=(cp == 0),
                        stop=(cp == 1),
                    )
            xsT = sp.tile([P, 8], F32R)
            nc.vector.tensor_copy(xsT[:], ps_xsT[:])

            # ---- Sfull row (1, 1024) = xs @ Wv  (wide fp32r, chases Wv DMA)
            ps_S = psS.tile([1, E], F32, tag="pss")
            for c in range(8):
                base = (c % 2) * E
                for half in range(2):
                    sl = slice(half * 512, half * 512 + 512)
                    nc.tensor.matmul(
                        ps_S[0:1, sl],
                        xsT[:, c : c + 1],
                        wvt[c // 2][:, base + half * 512 : base + half * 512 + 512],
                        start=(c == 0),
                        stop=(c == 7),
                        skip_group_check=True,
                    )
            srow = sp.tile([1, E], F32)
            nc.vector.tensor_copy(srow[:], ps_S[:])

            # ---- sft[d, h] = Sfull[64h + d]  (N=1 fp32, base partition 0)
            ps_sft = psA.tile([D, H], F32, tag="psa")
            for h in range(H):
                nc.tensor.matmul(
                    ps_sft[:, h : h + 1],
                    srow[0:1, h * D : (h + 1) * D],
                    ones_sb[0:1, 0:1],
                    start=True,
                    stop=True,
                )
            sft = sp.tile([D, H], F32)
            nc.vector.tensor_copy(sft[:], ps_sft[:])

            # ---- rep[d, 64h + r] = sft[d, h]  (DVE free-dim broadcast, fp32r out)
            rep = sp.tile([D, H * D], F32R)
            nc.vector.tensor_copy(
                rep[:].rearrange("d (h r) -> d h r", r=D),
                sft[:, :, None].to_broadcast((D, H, D)),
            )

            # ---- WoSum[d, :] = sum_m Wo[64m + d, :]: PE fold with double
            #      identity, chasing the Wo DMAs (wide fp32r)
            ps_fold = psF.tile([D, E], F32, tag="psf")
            for i in range(4):
                for cp in range(2):
                    k = 2 * i + cp
                    for half in range(2):
                        sl = slice(half * 512, half * 512 + 512)
                        nc.tensor.matmul(
                            ps_fold[:, sl],
                            dbl_r[:],
                            wot[i][:, cp * E + half * 512 : cp * E + half * 512 + 512],
                            start=(k == 0),
                            stop=(k == 7),
                            skip_group_check=True,
                        )
            wosum = sp.tile([D, E], F32R)
            nc.vector.tensor_copy(wosum[:], ps_fold[:])

            # ---- fused T+broadcast, two 128-row blocks per 1 MB output DMA
            outr = outd.rearrange("(i c p) e -> i p c e", p=P, c=2)
            for i in range(4):
                ob = op.tile([P, 2 * E], F32)
                for c in range(2):
                    t = 2 * i + c
                    for half in range(2):
                        sl = slice(half * 512, half * 512 + 512)
                        po = psO.tile([P, 512], F32, tag="pso")
                        nc.tensor.matmul(
                            po[:],
                            rep[:, t * P : (t + 1) * P],
                            wosum[:, sl],
                            start=True,
                            stop=True,
                        )
                        # bias add fused with PSUM->SBUF move
                        nc.vector.tensor_add(
                            ob[:, c * E + half * 512 : c * E + half * 512 + 512],
                            po[:],
                            bo_sb[:, sl],
                        )
                dmae[i % 2].dma_start(
                    out=outr[i], in_=ob[:].rearrange("p (c e) -> p c e", c=2)
                )

    nc.compile()
    return nc


_NC_CACHE = None


def make_in_maps(x, Wv, Wo, bo):
    x = np.ascontiguousarray(np.asarray(x, dtype=np.float32))
    Wv = np.ascontiguousarray(np.asarray(Wv, dtype=np.float32))
    Wo = np.ascontiguousarray(np.asarray(Wo, dtype=np.float32))
    bo = np.ascontiguousarray(np.asarray(bo, dtype=np.float32))
    bo128 = np.tile(bo.reshape(1, E), (P, 1))
    ones128 = np.ones((P, 1), dtype=np.float32)
    dblI = np.zeros((P, D), dtype=np.float32)
    dblI[np.arange(P), np.arange(P) % D] = 1.0
    return [
        {
            "x": np.ascontiguousarray(x[j]),
            "Wv": Wv,
            "Wo": Wo,
            "bo128": bo128,
            "ones128": ones128,
            "dblI": dblI,
        }
        for j in range(NCORES)
    ]


def kernel(x, Wq=None, Wk=None, Wv=None, Wo=None, bo=None, **_unused):
    from concourse.bass_utils import run_bass_kernel_spmd

    global _NC_CACHE
    if _NC_CACHE is None:
        _NC_CACHE = build_nc()
    nc = _NC_CACHE

    in_maps = make_in_maps(x, Wv, Wo, bo)
    res = run_bass_kernel_spmd(nc, in_maps, core_ids=list(range(NCORES))).results
    return np.stack([res[j]["out"] for j in range(NCORES)], axis=0)



# revision 15
# speedup vs baseline: 1.2606x; 1.2606x over previous
"""Trainium2 Bass kernel for nn_MultiHeadAttention_79130477461654.

The reference einsum "nhqk,nhvd->nhqd" contracts k and v independently, so
out = (sum_k softmax(energy))*(sum_s v) = broadcast(sum_s v) since softmax
rows sum to 1.  With v = split_heads(x @ Wv) and the reference's direct
(n,h,q,d)->(n,s,e) reshape, the full output reduces to

    xs[n]    = sum_s x[n,s,:]                       (1024,)
    Sfull[n] = xs[n] @ Wv                           (1024,)
    T[n,h,:] = tile16(Sfull[n][64h:64h+64]) @ Wo + bo   (16, 1024)
    out[n, 64h+r, :] = T[n,h,:]   for r in 0..63

Sharding: data parallel over batch N=8, one batch per core; Wv/Wo
replicated.  All tensors are shipped to the device in bf16 (host-side
dtype cast only; every reduction/matmul runs on-device with f32 PSUM
accumulation; end-to-end rel err ~5e-3 vs the 2e-2 gate).  Per-core HBM
traffic: x 2MB + Wv 2MB + Wo 2MB in, out 2MB.

Pipeline: x streams first (4 transfers split over both HWDGE queues)
and reduces on DVE chasing the DMAs; a dozen dummy matmuls warm the PE
HAM clock gate meanwhile.  S = xs @ Wv chases the Wv stream.  The
second GEMM accumulates T128[8h+j, :] = T[h, :] directly: its
stationary operand dbl128[p, 8h+j] = Sfull[64h + p%64] replicates each
head's column 8x so the product covers all 128 partitions.  Wo arrives
column-split (two 1MB halves), so T's first column half and its eight
256KB output DMAs overlap the second half's stream; each output DMA
writes the plain [128, 512] T128 tile to a strided DRAM row view
(row h*64 + 8*ra + rb takes partition 8h + rb).  The bias enters the
PSUM accumulation as a ones_row (x) bo rank-1 matmul.
"""

import numpy as np

N, S, E, H, D = 8, 1024, 1024, 16, 64
NCORES = 8
P = 128  # partitions
REP = P // H  # 8 copies of each head row
HF = 512  # column half


def build_nc():
    import concourse.bacc as bacc
    import concourse.mybir as mybir
    from concourse.tile import TileContext

    F32 = mybir.dt.float32
    BF16 = mybir.dt.bfloat16
    nc = bacc.Bacc("TRN2", target_bir_lowering=False, debug=False)

    xd = nc.declare_dram_parameter("x", [S, E], BF16, isOutput=False)
    wvd = nc.declare_dram_parameter("Wv", [E, E], BF16, isOutput=False)
    wod = nc.declare_dram_parameter("Wo", [E, E], BF16, isOutput=False)
    bod = nc.declare_dram_parameter("bo1", [1, E], BF16, isOutput=False)
    ones128d = nc.declare_dram_parameter("ones128", [P, 1], BF16, isOutput=False)
    seld = nc.declare_dram_parameter("sel", [P, 8 * P], BF16, isOutput=False)
    outd = nc.declare_dram_parameter("out", [S, E], BF16, isOutput=True)

    # two HWDGE queues: SP (sync) and ACT (scalar); ACT issues DMAs only
    dmae = [nc.sync, nc.scalar]

    with TileContext(nc) as tc:
        with (
            tc.tile_pool(name="xin", bufs=4) as xp,
            tc.tile_pool(name="wv", bufs=2) as wvp,
            tc.tile_pool(name="wo", bufs=2) as wop,
            tc.tile_pool(name="small", bufs=1) as sp,
            tc.tile_pool(name="psW", bufs=1, space="PSUM") as psW,
            tc.tile_pool(name="psA", bufs=1, space="PSUM") as psA,
            tc.tile_pool(name="psS", bufs=1, space="PSUM") as psS,
            tc.tile_pool(name="psT", bufs=1, space="PSUM") as psT,
            tc.tile_pool(name="psO", bufs=2, space="PSUM") as psO,
        ):
            ones_sb = sp.tile([P, 1], BF16)
            dmae[0].dma_start(out=ones_sb[:], in_=ones128d[:])
            bo_sb = sp.tile([1, E], BF16)
            dmae[1].dma_start(out=bo_sb[:], in_=bod[:])

            # ---- x: 4 x 512KB paired transfers (rows (2i+c)*128+p), heads
            #      of both queues so the stream lands before the weights.
            xt = []
            for i in range(4):
                t = xp.tile([P, 2 * E], BF16)
                dmae[i % 2].dma_start(
                    out=t[:].rearrange("p (c e) -> p c e", c=2),
                    in_=xd.rearrange("(i c p) e -> i p c e", p=P, c=2)[i],
                )
                xt.append(t)
            # Wv row-split (2 x 1MB), Wo column-split (two 1MB halves so the
            # second GEMM + output can start after the first half lands).
            wvt = []
            for i in range(2):
                t = wvp.tile([P, 4 * E], BF16, tag="wv")
                dmae[i % 2].dma_start(
                    out=t[:].rearrange("p (c e) -> p c e", c=4),
                    in_=wvd.rearrange("(i c p) e -> i p c e", p=P, c=4)[i],
                )
                wvt.append(t)
            wog = []
            for g in range(2):
                t = wop.tile([P, 8 * HF], BF16, tag="wo")
                dmae[g % 2].dma_start(
                    out=t[:].rearrange("p (c e) -> p c e", c=8),
                    in_=wod.rearrange("(c p) (g e) -> g p c e", p=P, e=HF)[g],
                )
                wog.append(t)
            sel_sb = sp.tile([P, 8 * P], BF16, tag="sel")
            dmae[0].dma_start(out=sel_sb[:], in_=seld[:])

            # ---- PE warm-up: ~7us of dummy activity against the ones
            #      column flips the HAM clock gate to 2.4 GHz and keeps it
            #      there until the real matmul chain starts.
            ps_warm = psW.tile([1, 512], F32, tag="psw")
            for _ in range(12):
                nc.tensor.matmul(
                    ps_warm[0:1, :],
                    ones_sb[:],
                    ones_sb[:, 0:1].to_broadcast((P, 512)),
                    start=True,
                    stop=True,
                )

            # ---- DVE reduction tree over the 8 row-chunks of x (bf16 2x)
            af = []
            for i in range(4):
                t = sp.tile([P, E], BF16, tag=f"af{i}")
                nc.vector.tensor_add(t[:], xt[i][:, 0:E], xt[i][:, E : 2 * E])
                af.append(t)
            a01 = sp.tile([P, E], BF16, tag="a01")
            nc.vector.tensor_add(a01[:], af[0][:], af[1][:])
            a23 = sp.tile([P, E], BF16, tag="a23")
            nc.vector.tensor_add(a23[:], af[2][:], af[3][:])
            xacc = sp.tile([P, E], BF16, tag="xacc")
            nc.vector.tensor_add(xacc[:], a01[:], a23[:])

            # ---- xsT[p, c] = xs[128c + p]: PE partition reduction of xacc.
            ps_xsT = psA.tile([P, 8], F32, tag="psa")
            for c in range(8):
                nc.tensor.matmul(
                    ps_xsT[:, c : c + 1],
                    xacc[:, c * P : (c + 1) * P],
                    ones_sb[:],
                    start=True,
                    stop=True,
                )
            xsT_b = sp.tile([P, 8], BF16, tag="xsT")
            nc.vector.tensor_copy(xsT_b[:], ps_xsT[:])

            # ---- Sfull row (1, 1024) = xs @ Wv (bf16, chases Wv DMAs)
            ps_S = psS.tile([1, E], F32, tag="pss")
            for k in range(8):
                i, c = k // 4, k % 4
                for half in range(2):
                    sl = slice(half * HF, half * HF + HF)
                    nc.tensor.matmul(
                        ps_S[0:1, sl],
                        xsT_b[:, k : k + 1],
                        wvt[i][:, c * E + half * HF : c * E + half * HF + HF],
                        start=(k == 0),
                        stop=(k == 7),
                        skip_group_check=True,
                    )
            srow_b = sp.tile([1, E], BF16, tag="srow")
            nc.vector.tensor_copy(srow_b[:], ps_S[:])

            # ---- dbl[p, h] = Sfull[64h + p%64] (N=1 matmuls, disjoint
            #      start/stop groups), then replicate each head column 8x.
            ps_dbl = psA.tile([P, H], F32, tag="psa")
            for q in range(2):
                for h in range(H):
                    nc.tensor.matmul(
                        ps_dbl[q * D : (q + 1) * D, h : h + 1],
                        srow_b[0:1, h * D : (h + 1) * D],
                        ones_sb[0:1, 0:1],
                        start=True,
                        stop=True,
                    )
            dbl_b = sp.tile([P, H], BF16, tag="dbl")
            nc.vector.tensor_copy(dbl_b[:], ps_dbl[:])
            dbl128 = sp.tile([P, P], BF16, tag="dbl128")
            nc.vector.tensor_copy(
                dbl128[:].rearrange("p (h j) -> p h j", j=REP),
                dbl_b[:, :, None].to_broadcast((P, H, REP)),
            )
            ones_r = sp.tile([1, P], BF16, tag="onesr")
            nc.vector.tensor_copy(ones_r[:], ones_sb[0:1, 0:1].to_broadcast((1, P)))

            # ---- T128[8h+j, :] = T[h, :] = dbl128.T @ Wo + bo, one column
            #      half at a time (chases the Wo halves); bias opens each
            #      half's accumulation as a ones_row (x) bo rank-1 matmul.
            #      Each finished half expands to full 1024-row form on the
            #      PE (sel one-hot picks T128 partition 16j + 8*(m//64) for
            #      output chunk j) and streams out as two 512KB DMAs.
            ps_T = psT.tile([P, E], F32, tag="pst")
            T_sb = sp.tile([P, E], BF16, tag="tsb")
            outr = outd.rearrange("(j p) (g e) -> g p j e", p=P, e=HF)
            for g in range(2):
                sl = slice(g * HF, g * HF + HF)
                nc.tensor.matmul(
                    ps_T[:, sl],
                    ones_r[:],
                    bo_sb[0:1, sl],
                    start=True,
                    stop=False,
                    skip_group_check=True,
                )
                for k in range(8):
                    nc.tensor.matmul(
                        ps_T[:, sl],
                        dbl128[:],
                        wog[g][:, k * HF : (k + 1) * HF],
                        start=False,
                        stop=(k == 7),
                        skip_group_check=True,
                    )
                nc.vector.tensor_copy(T_sb[:, sl], ps_T[:, sl])
                ob = sp.tile([P, 8 * HF], BF16, tag=f"ob{g}")
                for j in range(8):
                    po = psO.tile([P, HF], F32, tag="pso")
                    nc.tensor.matmul(
                        po[:],
                        sel_sb[:, j * P : (j + 1) * P],
                        T_sb[:, sl],
                        start=True,
                        stop=True,
                    )
                    nc.vector.tensor_copy(ob[:, j * HF : (j + 1) * HF], po[:])
                for jj in range(2):
                    dmae[jj].dma_start(
                        out=outr[g][:, jj * 4 : (jj + 1) * 4, :],
                        in_=ob[:, jj * 4 * HF : (jj + 1) * 4 * HF].rearrange(
                            "p (j e) -> p j e", j=4
                        ),
                    )

    nc.compile()
    return nc


_NC_CACHE = None


def make_in_maps(x, Wv, Wo, bo):
    import ml_dtypes

    bf16 = ml_dtypes.bfloat16
    x = np.asarray(x).astype(bf16)
    Wv = np.ascontiguousarray(np.asarray(Wv).astype(bf16))
    Wo = np.ascontiguousarray(np.asarray(Wo).astype(bf16))
    bo1 = np.asarray(bo).astype(bf16).reshape(1, E)
    ones128 = np.ones((P, 1), dtype=bf16)
    sel = np.zeros((P, 8 * P), dtype=np.float32)
    for j in range(8):
        for m in range(P):
            sel[16 * j + 8 * (m // D), j * P + m] = 1.0
    sel = sel.astype(bf16)
    return [
        {
            "x": np.ascontiguousarray(x[j]),
            "Wv": Wv,
            "Wo": Wo,
            "bo1": bo1,
            "ones128": ones128,
            "sel": sel,
        }
        for j in range(NCORES)
    ]


def kernel(x, Wq=None, Wk=None, Wv=None, Wo=None, bo=None, **_unused):
    from concourse.bass_utils import run_bass_kernel_spmd

    global _NC_CACHE
    if _NC_CACHE is None:
        _NC_CACHE = build_nc()
    nc = _NC_CACHE

    in_maps = make_in_maps(x, Wv, Wo, bo)
    res = run_bass_kernel_spmd(nc, in_maps, core_ids=list(range(NCORES))).results
    return np.stack(
        [np.asarray(res[j]["out"]).astype(np.float32) for j in range(NCORES)], axis=0
    )


# revision 22
# speedup vs baseline: 1.3924x; 1.1045x over previous
"""Trainium2 Bass kernel for nn_MultiHeadAttention_79130477461654.

The reference einsum "nhqk,nhvd->nhqd" contracts k and v independently, so
out = (sum_k softmax(energy))*(sum_s v) = broadcast(sum_s v) since softmax
rows sum to 1.  With v = split_heads(x @ Wv) and the reference's direct
(n,h,q,d)->(n,s,e) reshape, the full output reduces to

    xs[n]    = sum_s x[n,s,:]                       (1024,)
    Sfull[n] = xs[n] @ Wv                           (1024,)
    T[n,h,:] = tile16(Sfull[n][64h:64h+64]) @ Wo + bo   (16, 1024)
    out[n, 64h+r, :] = T[n,h,:]   for r in 0..63

Sharding: data parallel over batch N=8, one batch per core; Wv/Wo
replicated.  All tensors ship in bf16 and are pre-shuffled on the host
into the exact SBUF tile layouts (pure layout transform: every DMA line
is >=4KB contiguous, ~420GB/s).  All arithmetic runs on-device with f32
PSUM accumulation; end-to-end rel err ~5e-3 vs the 2e-2 gate.  Per-core
HBM traffic: x 2MB + Wv 2MB + Wo 2MB in, out 2MB.

Pipeline: x streams first on both HWDGE queues and reduces on DVE
chasing the DMAs while dummy matmuls warm the PE HAM clock gate; S
accumulates 1024-wide matmuls in wv-arrival order; dbl128[p, 8h+j] =
Sfull[64h + p%64] makes T128[8h+j, :] = T[h, :] land on all 128
partitions; the output expands to full 1024-row form with one-hot sel
matmuls (PSUM alternating between the psO pool and the retired psT
bank) and PSUM->SBUF copies alternating DVE/ACT, then streams out as
four 512KB DMAs.
"""

import numpy as np

N, S, E, H, D = 8, 1024, 1024, 16, 64
NCORES = 8
P = 128  # partitions
REP = P // H  # 8 copies of each head row


def build_nc():
    import concourse.bacc as bacc
    import concourse.mybir as mybir
    from concourse.tile import TileContext

    F32 = mybir.dt.float32
    BF16 = mybir.dt.bfloat16
    nc = bacc.Bacc("TRN2", target_bir_lowering=False, debug=False)

    xd = nc.declare_dram_parameter("x", [P, 8 * E], BF16, isOutput=False)
    wvd = nc.declare_dram_parameter("Wv", [P, 8 * E], BF16, isOutput=False)
    wod = nc.declare_dram_parameter("Wo", [P, 8 * E], BF16, isOutput=False)
    bod = nc.declare_dram_parameter("bo1", [1, E], BF16, isOutput=False)
    ones128d = nc.declare_dram_parameter("ones128", [P, 1], BF16, isOutput=False)
    seld = nc.declare_dram_parameter("sel", [P, 8 * P], BF16, isOutput=False)
    outd = nc.declare_dram_parameter("out", [S, E], BF16, isOutput=True)

    # two HWDGE queues: SP (sync) and ACT (scalar)
    dmae = [nc.sync, nc.scalar]
    # wv/wo quarter-transfer completion order given the queue layout below:
    # q0 gets chunk-pairs (0,1) then (2,3); q1 gets (4,5) then (6,7).
    KORDER = [0, 1, 4, 5, 2, 3, 6, 7]

    with TileContext(nc) as tc:
        with (
            tc.tile_pool(name="xin", bufs=4) as xp,
            tc.tile_pool(name="wv", bufs=4) as wvp,
            tc.tile_pool(name="wo", bufs=4) as wop,
            tc.tile_pool(name="small", bufs=1) as sp,
            tc.tile_pool(name="psA", bufs=1, space="PSUM") as psA,
            tc.tile_pool(name="psS", bufs=1, space="PSUM") as psS,
            tc.tile_pool(name="psT", bufs=1, space="PSUM") as psT,
            tc.tile_pool(name="psO", bufs=1, space="PSUM") as psO,
        ):
            ones_sb = sp.tile([P, 1], BF16)
            dmae[0].dma_start(out=ones_sb[:], in_=ones128d[:])
            bo_sb = sp.tile([1, E], BF16)
            dmae[1].dma_start(out=bo_sb[:], in_=bod[:])

            # ---- x: 4 x 512KB transfers, heads of both queues.  Transfer i
            #      holds rows [256i, 256i+256): line p = rows 256i+2p(+1).
            xt = []
            for i in range(4):
                t = xp.tile([P, 2 * E], BF16)
                dmae[i % 2].dma_start(
                    out=t[:], in_=xd[:, i * 2 * E : (i + 1) * 2 * E]
                )
                xt.append(t)
            # Wv/Wo: 4 x 512KB each, chunk-pairs (0,1)/(2,3) on q0 and
            # (4,5)/(6,7) on q1, so halves of both land early; consumers
            # run in KORDER.  wv tile q holds K-chunks 2q, 2q+1 of Wv;
            # wo tile q likewise (full-width rows).
            wvt, wot = [], []
            for q in range(4):
                pair = [0, 2, 1, 3][q]  # issue order: (0,1) q0, (4,5) q1, ...
                t = wvp.tile([P, 2 * E], BF16, tag=f"wv{pair}")
                dmae[q % 2].dma_start(
                    out=t[:], in_=wvd[:, pair * 2 * E : (pair + 1) * 2 * E]
                )
                wvt.append((pair, t))
            for q in range(4):
                pair = [0, 2, 1, 3][q]
                t = wop.tile([P, 2 * E], BF16, tag=f"wo{pair}")
                dmae[q % 2].dma_start(
                    out=t[:], in_=wod[:, pair * 2 * E : (pair + 1) * 2 * E]
                )
                wot.append((pair, t))
            sel_sb = sp.tile([P, 8 * P], BF16, tag="sel")
            dmae[0].dma_start(out=sel_sb[:], in_=seld[:])
            wvmap = {pair: t for pair, t in wvt}
            womap = {pair: t for pair, t in wot}

            # ---- PE warm-up: dummy 256-wide matmuls flip the HAM clock
            #      gate to 2.4 GHz and keep it there until xsT/S start.
            ps_warm = psA.tile([1, 256], F32, tag="psw")
            for _ in range(18):
                nc.tensor.matmul(
                    ps_warm[0:1, :],
                    ones_sb[:],
                    ones_sb[:, 0:1].to_broadcast((P, 256)),
                    start=True,
                    stop=True,
                )

            # ---- DVE reduction tree over the 8 row-chunks of x (bf16 2x)
            af = []
            for i in range(4):
                t = sp.tile([P, E], BF16, tag=f"af{i}")
                nc.vector.tensor_add(t[:], xt[i][:, 0:E], xt[i][:, E : 2 * E])
                af.append(t)
            a01 = sp.tile([P, E], BF16, tag="a01")
            nc.vector.tensor_add(a01[:], af[0][:], af[1][:])
            a23 = sp.tile([P, E], BF16, tag="a23")
            nc.vector.tensor_add(a23[:], af[2][:], af[3][:])
            xacc = sp.tile([P, E], BF16, tag="xacc")
            nc.vector.tensor_add(xacc[:], a01[:], a23[:])

            # ---- xsT[p, c] = xs[128c + p]: PE partition reduction of xacc.
            ps_xsT = psA.tile([P, 8], F32, tag="psa")
            for c in range(8):
                nc.tensor.matmul(
                    ps_xsT[:, c : c + 1],
                    xacc[:, c * P : (c + 1) * P],
                    ones_sb[:],
                    start=True,
                    stop=True,
                )
            xsT_b = sp.tile([P, 8], BF16, tag="xsT")
            nc.vector.tensor_copy(xsT_b[:], ps_xsT[:])

            # ---- Sfull row (1, 1024) = xs @ Wv: 1024-wide matmuls in
            #      wv-arrival order.
            ps_S = psS.tile([1, E], F32, tag="pss")
            for idx, k in enumerate(KORDER):
                for half in range(2):
                    sl = slice(half * 512, half * 512 + 512)
                    nc.tensor.matmul(
                        ps_S[0:1, sl],
                        xsT_b[:, k : k + 1],
                        wvmap[k // 2][
                            :, (k % 2) * E + half * 512 : (k % 2) * E + half * 512 + 512
                        ],
                        start=(idx == 0),
                        stop=(idx == 7),
                        skip_group_check=True,
                    )
            srow_b = sp.tile([1, E], BF16, tag="srow")
            nc.vector.tensor_copy(srow_b[0:1, 0:512], ps_S[0:1, 0:512])
            nc.scalar.copy(out=srow_b[0:1, 512:E], in_=ps_S[0:1, 512:E])

            # ---- dbl[p, h] = Sfull[64h + p%64] (N=1 matmuls, disjoint
            #      start/stop groups), then replicate each head column 8x.
            ps_dbl = psA.tile([P, H], F32, tag="psa")
            for q in range(2):
                for h in range(H):
                    nc.tensor.matmul(
                        ps_dbl[q * D : (q + 1) * D, h : h + 1],
                        srow_b[0:1, h * D : (h + 1) * D],
                        ones_sb[0:1, 0:1],
                        start=True,
                        stop=True,
                    )
            dbl_b = sp.tile([P, H], BF16, tag="dbl")
            nc.vector.tensor_copy(dbl_b[:], ps_dbl[:])
            dbl128 = sp.tile([P, P], BF16, tag="dbl128")
            nc.vector.tensor_copy(
                dbl128[:].rearrange("p (h j) -> p h j", j=REP),
                dbl_b[:, :, None].to_broadcast((P, H, REP)),
            )
            ones_r = sp.tile([1, P], BF16, tag="onesr")
            nc.vector.tensor_copy(ones_r[:], ones_sb[0:1, 0:1].to_broadcast((1, P)))

            # ---- T128[8h+j, :] = T[h, :] = dbl128.T @ Wo + bo: 1024-wide
            #      matmuls in wo-arrival order; bias opens the group.
            ps_T = psT.tile([P, E], F32, tag="pst")
            for half in range(2):
                sl = slice(half * 512, half * 512 + 512)
                nc.tensor.matmul(
                    ps_T[:, sl],
                    ones_r[:],
                    bo_sb[0:1, sl],
                    start=True,
                    stop=False,
                    skip_group_check=True,
                )
            for idx, k in enumerate(KORDER):
                for half in range(2):
                    sl = slice(half * 512, half * 512 + 512)
                    nc.tensor.matmul(
                        ps_T[:, sl],
                        dbl128[:],
                        womap[k // 2][
                            :, (k % 2) * E + half * 512 : (k % 2) * E + half * 512 + 512
                        ],
                        start=False,
                        stop=(idx == 7),
                        skip_group_check=True,
                    )
            T_sb = sp.tile([P, E], BF16, tag="tsb")
            nc.vector.tensor_copy(T_sb[:, 0:512], ps_T[:, 0:512])
            nc.scalar.copy(out=T_sb[:, 512:E], in_=ps_T[:, 512:E])

            # ---- expansion: out chunk j rows 128j+p = T[2j + p//64] via
            #      one-hot sel matmuls (PSUM alternates psO pool / retired
            #      psT bank; copies alternate DVE/ACT), then 4 x 512KB out.
            ob = sp.tile([P, 8 * E], BF16, tag="ob")
            outr = outd.rearrange("(jj c p) e -> jj p c e", c=2, p=P)
            for j in range(8):
                if j % 2 == 0:
                    po = psO.tile([P, E], F32, tag="pso")
                else:
                    po = psT.tile([P, E], F32, tag="pst")
                for half in range(2):
                    sl = slice(half * 512, half * 512 + 512)
                    nc.tensor.matmul(
                        po[:, sl],
                        sel_sb[:, j * P : (j + 1) * P],
                        T_sb[:, sl],
                        start=True,
                        stop=True,
                    )
                if j % 2 == 0:
                    nc.vector.tensor_copy(ob[:, j * E : (j + 1) * E], po[:])
                else:
                    nc.scalar.copy(out=ob[:, j * E : (j + 1) * E], in_=po[:])
                if j % 2 == 1:
                    jj = j // 2
                    dmae[jj % 2].dma_start(
                        out=outr[jj],
                        in_=ob[:, jj * 2 * E : (jj + 1) * 2 * E].rearrange(
                            "p (c e) -> p c e", c=2
                        ),
                    )

    nc.compile()
    return nc


_NC_CACHE = None


def make_in_maps(x, Wv, Wo, bo):
    import ml_dtypes

    bf16 = ml_dtypes.bfloat16
    x = np.asarray(x).astype(bf16)
    Wv = np.asarray(Wv).astype(bf16)
    Wo = np.asarray(Wo).astype(bf16)
    # pre-shuffle into SBUF tile layouts (pure layout transforms):
    # x_pre[p, i*2048 + r*1024 + e] = x[n, 256i + 2p + r, e]
    xs_pre = [
        np.ascontiguousarray(
            x[j].reshape(4, P, 2, E).transpose(1, 0, 2, 3).reshape(P, 8 * E)
        )
        for j in range(NCORES)
    ]
    # w_pre[p, k*1024 + e] = W[128k + p, e]
    wv_pre = np.ascontiguousarray(
        Wv.reshape(8, P, E).transpose(1, 0, 2).reshape(P, 8 * E)
    )
    wo_pre = np.ascontiguousarray(
        Wo.reshape(8, P, E).transpose(1, 0, 2).reshape(P, 8 * E)
    )
    bo1 = np.asarray(bo).astype(bf16).reshape(1, E)
    ones128 = np.ones((P, 1), dtype=bf16)
    sel = np.zeros((P, 8 * P), dtype=np.float32)
    for j in range(8):
        for m in range(P):
            sel[16 * j + 8 * (m // D), j * P + m] = 1.0
    sel = sel.astype(bf16)
    return [
        {
            "x": xs_pre[j],
            "Wv": wv_pre,
            "Wo": wo_pre,
            "bo1": bo1,
            "ones128": ones128,
            "sel": sel,
        }
        for j in range(NCORES)
    ]


def kernel(x, Wq=None, Wk=None, Wv=None, Wo=None, bo=None, **_unused):
    from concourse.bass_utils import run_bass_kernel_spmd

    global _NC_CACHE
    if _NC_CACHE is None:
        _NC_CACHE = build_nc()
    nc = _NC_CACHE

    in_maps = make_in_maps(x, Wv, Wo, bo)
    res = run_bass_kernel_spmd(nc, in_maps, core_ids=list(range(NCORES))).results
    return np.stack(
        [np.asarray(res[j]["out"]).astype(np.float32) for j in range(NCORES)], axis=0
    )
